# revision 52
# baseline (speedup 1.0000x reference)
"""GCN (2x GCNConv + BN + ReLU + FC) on 8 Trainium2 NeuronCores — v3.

Feature-major dataflow, one bf16 AllGather, single-window L2 gather.

Math: per GCNConv, z[d] = dinv_d * sum_{s->d} dinv_s * h[s] (+bias absorbed
by BN). Structure:
  - L1: columns (edge slots) are host-relayouted from x (edge-ordered,
    dinv_src-scaled, bf16) and streamed; segment-sum fused into the W1
    matmul via PSUM accumulation over slabs.
  - L2: gather bf16 h1' columns out of the AllGathered gout with ONE
    dma_gather window: int16 indices encode (src_row - 17408), exploiting
    sign-extended index address math to cover all 50176 rows.
  - dests sorted by degree within each core so per-128-block max-degree
    padding is minimal; K uniform per block -> one contiguous DVE reduce
    [128, 128, K] per block.
  - BN affine per-partition on ScalarE; BN stats via 1KB AllReduce.

Host does layout/normalization-constant work only (edge ordering, dinv,
tables); all FP reductions/matmuls/BN on device.
"""

import numpy as np
import ml_dtypes

N, E, D, H, O = 50000, 600000, 128, 128, 64
BN_EPS = 1e-5
NC = 8
P = 128
BPC = 49
SHARD = BPC * P          # 6272
NPAD = NC * SHARD        # 50176
OFF = NPAD - 32768       # 17408: idx int16 = row - OFF in [-17408, 32767]
C_COLS = 4096            # target gather cols per dma_gather call


def _wrap_idx(flat):
    flat = np.asarray(flat, np.int16)
    assert flat.size % 16 == 0
    w16 = flat.reshape(-1, 16).T.copy()
    return np.tile(w16, (8, 1))          # [128, n/16]


class Plan:
    pass


def build_plan(edge_index: np.ndarray) -> Plan:
    pl = Plan()
    row = edge_index[0].astype(np.int64)
    col = edge_index[1].astype(np.int64)
    loops = np.arange(N, dtype=np.int64)
    row = np.concatenate([row, loops])
    col = np.concatenate([col, loops])

    deg = np.bincount(col, minlength=N)
    dinv = (1.0 / np.sqrt(deg)).astype(np.float64)
    pl.dinv = dinv.astype(np.float32)

    order = np.argsort(-deg, kind="stable")
    # node -> core (snake deal by degree for edge balance)
    core_of = np.zeros(N, np.int64)
    snake = np.tile(np.r_[np.arange(NC), np.arange(NC)[::-1]],
                    (N + 2 * NC - 1) // (2 * NC))[:N]
    core_of[order] = snake

    # per-core dest ordering: plain degree desc -> tight per-block max-K
    node_of_agrow = np.full(NPAD, -1, np.int64)
    for c in range(NC):
        nodes = np.flatnonzero(core_of == c)
        srt = nodes[np.argsort(-deg[nodes], kind="stable")]
        node_of_agrow[c * SHARD:c * SHARD + len(srt)] = srt
    agrow_of_node = np.full(N, -1, np.int64)
    m = node_of_agrow >= 0
    agrow_of_node[node_of_agrow[m]] = np.flatnonzero(m)
    assert (agrow_of_node >= 0).all()

    # dummy (zero) row with agrow >= OFF for padding slots
    dummies = np.flatnonzero(~m)
    dums_hi = dummies[dummies >= OFF]
    assert len(dums_hi) > 0
    dummy_agrow = int(dums_hi[0])

    # block schedules (uniform across cores):
    #   K1[l] = max WITH-loop degree (L1 xeT slab schedule)
    #   K2[l] = max NON-loop degree (L2 gather schedule) — the appended
    #   self-loop contribution is each core's own upT tile, already resident
    #   feature-major in SBUF; gathering it would waste ~6272 Q7 descriptors
    #   per core. It is added as one extra W2 matmul per block instead.
    ndeg = np.bincount(edge_index[1].astype(np.int64), minlength=N)
    nodes_blk = node_of_agrow.reshape(NC, BPC, P)
    K1 = np.zeros(BPC, np.int64)
    K2 = np.zeros(BPC, np.int64)
    for c in range(NC):
        for l in range(BPC):
            blk = nodes_blk[c, l]
            real = blk[blk >= 0]
            if len(real):
                K1[l] = max(K1[l], deg[real].max())
                K2[l] = max(K2[l], ndeg[real].max())
    base2 = np.concatenate([[0], np.cumsum(K2 * P)])
    pl.K2, pl.base2 = K2, base2
    pl.COLS2 = int(base2[-1])

    # per-block last-lane guard: the LAST column of each block must encode a
    # non-negative idx (>= OFF source) or the ucode drops it as trailing pad.
    # A lane-127 node with deg < K2[l] ends in a pad (dummy_agrow >= OFF). If
    # every node in a block has deg == K2[l], move a node with a >= OFF
    # source (or pad) to lane 127 and put that source last.
    # We handle this after edge tables are built (cheap per-block fixup).

    # L1 groups of 4 blocks sharing one 512-wide PSUM accumulation chain
    G1 = (BPC + 3) // 4
    K1g = np.array([int(K1[4 * g:4 * g + 4].max()) for g in range(G1)])
    W1g = np.array([min(4, BPC - 4 * g) * P for g in range(G1)])
    base1g = np.concatenate([[0], np.cumsum(K1g * W1g)])
    pl.G1, pl.K1g, pl.W1g, pl.base1g = G1, K1g, W1g, base1g
    pl.COLS1 = int(base1g[-1])

    # per-agrow column bases
    lidx = np.arange(NPAD) % SHARD // P              # block l of each agrow
    pidx = np.arange(NPAD) % P
    gidx = lidx // 4
    bidx = lidx % 4
    cb1 = base1g[gidx] + bidx * P + pidx             # L1 col base per agrow
    w1 = W1g[gidx]                                   # L1 col stride per agrow
    # L2 blocks are SLAB-major: column = base2[l] + slot*128 + lane, so each
    # 128-wide slab is a contiguous matmul rhs (W2-fused PSUM accumulation).
    cb2 = base2[lidx] + pidx                         # L2 col base per agrow

    # per-edge (L1, WITH self-loops): dest agrow, source node, rank
    e_dst = agrow_of_node[col]
    o2 = np.argsort(e_dst, kind="stable")
    d_s = e_dst[o2]
    src_node = row[o2]
    newd = np.r_[True, d_s[1:] != d_s[:-1]]
    gstart = np.flatnonzero(newd)
    gid = np.cumsum(newd) - 1
    k_all = np.arange(len(d_s)) - gstart[gid]        # rank within dest
    core_d = d_s // SHARD

    # per-edge (L2, WITHOUT the appended self-loops)
    rowE = edge_index[0].astype(np.int64)
    colE = edge_index[1].astype(np.int64)
    eE_dst = agrow_of_node[colE]
    oE = np.argsort(eE_dst, kind="stable")
    dE = eE_dst[oE]
    srcagE = agrow_of_node[rowE[oE]]
    newdE = np.r_[True, dE[1:] != dE[:-1]]
    gstartE = np.flatnonzero(newdE)
    gidE = np.cumsum(newdE) - 1
    kE = np.arange(len(dE)) - gstartE[gidE]
    coreE = dE // SHARD

    pl.xe_src = []
    pl.idx2 = []
    for c in range(NC):
        mC = core_d == c
        dC, kC = d_s[mC], k_all[mC]
        xs = np.full(pl.COLS1, -1, np.int64)
        xs[cb1[dC] + kC * w1[dC]] = src_node[mC]
        mE = coreE == c
        ix = np.full(pl.COLS2, dummy_agrow - OFF, np.int64)
        ix[cb2[dE[mE]] + kE[mE] * P] = srcagE[mE] - OFF
        # last-column guard per block: block l's last column is (slot K-1,
        # lane 127); lane 127's slots are strided by 128.
        for l in range(BPC):
            k = int(K2[l])
            if k == 0:
                continue
            lo = int(base2[l]) + 127
            seg = ix[lo:lo + k * P:P].copy()
            if seg[k - 1] >= 0:
                continue
            pos = np.flatnonzero(seg >= 0)
            if len(pos):
                j = int(pos[-1])
                seg[k - 1], seg[j] = seg[j], seg[k - 1]
                ix[lo:lo + k * P:P] = seg
                # xe table is unaffected (L1 slot order independent)
                continue
            raise AssertionError(
                f"core {c} block {l}: no non-negative idx for last column")
        assert ix[int(base2[BPC]) - 1] >= 0 if False else True
        pl.xe_src.append(xs)
        pl.idx2.append(_wrap_idx(ix))

    # gather call chunking: (start_block, n_blocks, col0, ncols)
    out = []
    l = 0
    while l < BPC:
        l0, c0 = l, int(base2[l])
        l += 1                              # always take one block
        while l < BPC and int(base2[l + 1]) - c0 <= C_COLS:
            l += 1
        out.append((l0, l - l0, c0, int(base2[l]) - c0))
    pl.chunks2 = out
    pl.MAXCH = max(nc_ for (_, _, _, nc_) in out)

    pl.node_of_agrow = node_of_agrow
    pl.agrow_of_node = agrow_of_node
    return pl


def per_core_inputs(pl: Plan, x: np.ndarray) -> list[dict]:
    xs_pre = (x.astype(np.float64) * pl.dinv.astype(np.float64)[:, None])
    xs_pre = xs_pre.astype(np.float32)
    maps = []
    for c in range(NC):
        src = pl.xe_src[c]
        xe = np.zeros((pl.COLS1, D), np.float32)
        mm = src >= 0
        xe[mm] = xs_pre[src[mm]]
        dv = np.zeros(SHARD, np.float32)
        nodes = pl.node_of_agrow[c * SHARD:(c + 1) * SHARD]
        mn = nodes >= 0
        dv[mn] = pl.dinv[nodes[mn]]
        maps.append({
            "xeT": np.ascontiguousarray(xe.T).astype(ml_dtypes.bfloat16),
            "idx2": pl.idx2[c],
            "dinvb": dv.reshape(1, SHARD),
        })
    return maps


# ----------------------------------------------------------------------------
# Device program
# ----------------------------------------------------------------------------

def build_device(pl: Plan, debug=False):
    import concourse.bacc as bacc
    import concourse.mybir as mybir
    import concourse.tile as tile
    from concourse.masks import make_identity

    f32 = mybir.dt.float32
    bf16 = mybir.dt.bfloat16
    i16 = mybir.dt.int16
    Alu = mybir.AluOpType
    Act = mybir.ActivationFunctionType

    K2, base2 = pl.K2, pl.base2
    G1, K1g, W1g, base1g = pl.G1, pl.K1g, pl.W1g, pl.base1g
    COLS1, COLS2 = pl.COLS1, pl.COLS2

    nc = bacc.Bacc()

    xeT_in = nc.declare_dram_parameter("xeT", [P, COLS1], bf16, isOutput=False)
    idx2_in = nc.declare_dram_parameter("idx2", [P, COLS2 // 16], i16, isOutput=False)
    dinvb_in = nc.declare_dram_parameter("dinvb", [1, SHARD], f32, isOutput=False)
    W1_in = nc.declare_dram_parameter("W1", [D, H], bf16, isOutput=False)
    W2_in = nc.declare_dram_parameter("W2", [H, H], bf16, isOutput=False)
    Wfc_in = nc.declare_dram_parameter("Wfc", [H, O], bf16, isOutput=False)
    g1_in = nc.declare_dram_parameter("g1c", [H, 1], f32, isOutput=False)
    be1_in = nc.declare_dram_parameter("be1c", [H, 1], f32, isOutput=False)
    g2_in = nc.declare_dram_parameter("g2c", [H, 1], f32, isOutput=False)
    be2_in = nc.declare_dram_parameter("be2c", [H, 1], f32, isOutput=False)
    bfc_in = nc.declare_dram_parameter("bfcc", [O, 1], f32, isOutput=False)
    out_ext = nc.declare_dram_parameter("out", [O, SHARD], f32, isOutput=True)

    ag_in = nc.dram_tensor("ag_in", [SHARD, H], bf16)
    gout = nc.dram_tensor("gout", [NPAD, H], bf16, addr_space="Shared")
    ar_in = [nc.dram_tensor(f"ar_in{i}", [P, 2], f32) for i in range(2)]
    ar_out = [nc.dram_tensor(f"ar_out{i}", [P * NC, 2], f32, addr_space="Shared")
              for i in range(2)]
    rg = [list(range(NC))]

    with tile.TileContext(nc) as tc:
        with (
            tc.tile_pool(name="const", bufs=1) as cp,
            tc.tile_pool(name="xstg", bufs=2) as xp,
            tc.tile_pool(name="gstg", bufs=4) as gp,
            tc.tile_pool(name="big", bufs=1) as bigp,
            tc.tile_pool(name="tmp", bufs=2) as tp_,
            tc.tile_pool(name="scal", bufs=1) as sp,
        ):
            # ------- constants needed for L1 start (W1, idx2, dinv) -------
            W1s = cp.tile([D, H], bf16, tag="W1")
            nc.sync.dma_start(out=W1s[:], in_=W1_in[:])
            idx2 = cp.tile([P, COLS2 // 16], i16, tag="idx2")
            nc.sync.dma_start(out=idx2[:], in_=idx2_in[:])
            dinv_bc = cp.tile([P, SHARD], f32, tag="dinvbc")
            nc.sync.dma_start(out=dinv_bc[:], in_=dinvb_in[:].to_broadcast((P, SHARD)))

            u_T = bigp.tile([P, SHARD], f32, tag="u_T", name="u_T")
            gsrc = gout[OFF:OFF + 32768, :]

            # ------- gather-ucode warmup -------
            # The first dma_gather pays a ~10us ext-isa IRAM library load on
            # GpSimd. Issue a tiny throwaway gather NOW (gout content is
            # garbage pre-AllGather; addresses are in-bounds, output unused)
            # so the reload happens off the critical path.
            warm = sp.tile([P, 1, P], bf16, tag="warm")
            nc.gpsimd.dma_gather(
                out_ap=warm[:], in_ap=gsrc,
                idxs_ap=idx2[:, 0:8],
                num_idxs=P, num_idxs_reg=P, elem_size=H,
                transpose=True, single_packet=False)

            # ------- L1: fused aggregation+matmul via PSUM accumulation ----
            ps1 = tc.alloc_tile_pool(name="ps1", bufs=3, space="PSUM")
            SLABS_PER_STG = 16
            for g in range(G1):
                w = int(W1g[g])
                kg = int(K1g[g])
                c0 = int(base1g[g])
                mm = ps1.tile([P, w], f32, tag="mm1")
                for ks in range(0, kg, SLABS_PER_STG):
                    kn = min(SLABS_PER_STG, kg - ks)
                    xstg = xp.tile([P, SLABS_PER_STG * w], bf16, tag="xe")
                    nc.sync.dma_start(
                        out=xstg[:, 0:kn * w],
                        in_=xeT_in[:, c0 + ks * w:c0 + (ks + kn) * w])
                    for k in range(kn):
                        nc.tensor.matmul(
                            out=mm[:], lhsT=W1s[:],
                            rhs=xstg[:, k * w:(k + 1) * w],
                            start=(ks + k == 0), stop=(ks + k == kg - 1))
                o0 = g * 4 * P
                nc.vector.tensor_tensor(
                    out=u_T[:, o0:o0 + w], in0=mm[:], in1=dinv_bc[:, o0:o0 + w],
                    op=Alu.mult)
            ps1.release()

            # ------- remaining constants (first used after L1) -------
            W2s = cp.tile([H, H], bf16, tag="W2")
            Wfcs = cp.tile([H, O], bf16, tag="Wfc")
            nc.sync.dma_start(out=W2s[:], in_=W2_in[:])
            nc.sync.dma_start(out=Wfcs[:], in_=Wfc_in[:])
            gb = {}
            for nm, t in [("g1", g1_in), ("be1", be1_in),
                          ("g2", g2_in), ("be2", be2_in)]:
                gb[nm] = cp.tile([H, 1], f32, tag=nm, name=nm + "_sb")
                nc.sync.dma_start(out=gb[nm][:], in_=t[:])
            bfcs = cp.tile([O, 1], f32, tag="bfc")
            nc.sync.dma_start(out=bfcs[:], in_=bfc_in[:])
            ident = cp.tile([P, P], f32, tag="ident")
            make_identity(nc, ident[:])

            # BN stats helpers: per-chunk partials -> AllReduce -> affine
            def stats_chunk(sparts, i, c0, c1):
                sqc = tp_.tile([P, c1 - c0], f32, tag="sqc")
                nc.scalar.activation(out=sqc[:], in_=u_T[:, c0:c1],
                                     func=Act.Square, bias=0.0, scale=1.0)
                nc.vector.tensor_reduce(
                    out=sparts[:, 2 * i:2 * i + 1], in_=u_T[:, c0:c1],
                    axis=mybir.AxisListType.X, op=Alu.add)
                nc.vector.tensor_reduce(
                    out=sparts[:, 2 * i + 1:2 * i + 2], in_=sqc[:],
                    axis=mybir.AxisListType.X, op=Alu.add)

            def finish_stats(sparts, li):
                ssb = sp.tile([P, 2], f32, tag=f"ssb{li}")
                nc.vector.tensor_reduce(
                    out=ssb[:], in_=sparts[:].rearrange("p (c s) -> p s c", s=2),
                    axis=mybir.AxisListType.X, op=Alu.add)
                nc.sync.dma_start(out=ar_in[li][:], in_=ssb[:])
                # stats exchange as a tiny AllGather (lower floor + less
                # variance than AllReduce's reduce+broadcast phases); the
                # 8-shard sum is done locally on DVE.
                nc.gpsimd.collective_compute(
                    "AllGather", Alu.bypass, replica_groups=rg,
                    ins=[ar_in[li][:]], outs=[ar_out[li][:]])
                sall = sp.tile([P, 2 * NC], f32, tag=f"sall{li}")
                nc.sync.dma_start(
                    out=sall[:].rearrange("p (r s) -> p r s", s=2),
                    in_=ar_out[li][:].rearrange("(r p) s -> p r s", p=P))
                sums = sp.tile([P, 2], f32, tag=f"sums{li}")
                nc.vector.tensor_reduce(
                    out=sums[:],
                    in_=sall[:].rearrange("p (r s) -> p s r", s=2),
                    axis=mybir.AxisListType.X, op=Alu.add)
                mom = sp.tile([P, 2], f32, tag=f"mom{li}")
                nc.vector.tensor_scalar_mul(out=mom[:], in0=sums[:],
                                            scalar1=1.0 / N)
                msq = sp.tile([P, 1], f32, tag=f"msq{li}")
                nc.vector.tensor_tensor(out=msq[:], in0=mom[:, 0:1],
                                        in1=mom[:, 0:1], op=Alu.mult)
                v = sp.tile([P, 1], f32, tag=f"v{li}")
                nc.vector.tensor_tensor(out=v[:], in0=mom[:, 1:2], in1=msq[:],
                                        op=Alu.subtract)
                nc.vector.tensor_scalar_add(out=v[:], in0=v[:], scalar1=BN_EPS)
                sqv = sp.tile([P, 1], f32, tag=f"sqv{li}")
                nc.scalar.activation(out=sqv[:], in_=v[:], func=Act.Sqrt,
                                     bias=0.0, scale=1.0)
                rsq = sp.tile([P, 1], f32, tag=f"rsq{li}")
                nc.vector.reciprocal(out=rsq[:], in_=sqv[:])
                g, be = (gb["g1"], gb["be1"]) if li == 0 else (gb["g2"], gb["be2"])
                a = sp.tile([P, 1], f32, tag=f"a{li}")
                nc.vector.tensor_tensor(out=a[:], in0=rsq[:], in1=g[:], op=Alu.mult)
                ma = sp.tile([P, 1], f32, tag=f"ma{li}")
                nc.vector.tensor_tensor(out=ma[:], in0=mom[:, 0:1], in1=a[:],
                                        op=Alu.mult)
                B = sp.tile([P, 1], f32, tag=f"B{li}")
                nc.vector.tensor_tensor(out=B[:], in0=be[:], in1=ma[:],
                                        op=Alu.subtract)
                return a, B

            # L1 stats (u_T already computed by the fused PSUM path)
            nchs = (SHARD + 2047) // 2048
            sparts1 = sp.tile([P, 2 * nchs], f32, tag="sp0")
            for i in range(nchs):
                stats_chunk(sparts1, i, i * 2048, min((i + 1) * 2048, SHARD))
            a1, B1 = finish_stats(sparts1, 0)

            # h1 = relu(a*u + B); u' = h1 * dinv; transpose; ship — all
            # pipelined per 512-column chunk. upT aliases agg2's slot.
            # upB is a bf16 copy kept for the L2 self-loop term: each dest's
            # self contribution is its own u' column, already feature-major.
            upT = bigp.tile([P, SHARD], f32, tag="agg2", name="upT")
            upB = bigp.tile([P, SHARD], bf16, tag="upB", name="upB")
            pst = tc.alloc_tile_pool(name="pst", bufs=6, space="PSUM")
            for l0 in range(0, BPC, 4):
                nb = min(4, BPC - l0)
                csl = slice(l0 * P, (l0 + nb) * P)
                nc.scalar.activation(out=upT[:, csl], in_=u_T[:, csl],
                                     func=Act.Relu, bias=B1[:], scale=a1[:])
                nc.vector.tensor_tensor(out=upT[:, csl], in0=upT[:, csl],
                                        in1=dinv_bc[:, csl], op=Alu.mult)
                nc.vector.tensor_copy(out=upB[:, csl], in_=upT[:, csl])
                stg = tp_.tile([P, nb * P], bf16, tag="upstg")
                for j in range(nb):
                    l = l0 + j
                    tpm = pst.tile([P, P], f32, tag="tp")
                    nc.tensor.transpose(
                        out=tpm[:], in_=upT[:, l * P:(l + 1) * P],
                        identity=ident[:])
                    nc.scalar.activation(out=stg[:, j * P:(j + 1) * P],
                                         in_=tpm[:], func=Act.Copy,
                                         bias=0.0, scale=1.0)
                nc.sync.dma_start(
                    out=ag_in[l0 * P:(l0 + nb) * P, :].rearrange(
                        "(b p) q -> p b q", b=nb),
                    in_=stg[:].rearrange("p (b q) -> p b q", b=nb))
            pst.release()
            nc.gpsimd.collective_compute(
                "AllGather", Alu.bypass, replica_groups=rg,
                ins=[ag_in[:]], outs=[gout[:]])
            probe = sp.tile([1, H], bf16, tag="probe")
            nc.sync.dma_start(out=probe[:], in_=gout[0:1, :])

            # ---------------- L2 gather + W2-fused aggregation -------------
            # Segment-sum fused into the W2 matmul via PSUM accumulation over
            # 128-wide slabs (same trick as L1). Keeps the DVE idle during
            # descriptor generation — its bf16 2-port perf modes lock GpSimd
            # out of the SBUF state holding the SWDGE descriptor rings.
            sparts2 = sp.tile([P, 2 * BPC], f32, tag="sp1")
            ps2 = tc.alloc_tile_pool(name="ps2", bufs=3, space="PSUM")

            for ci, (l0, nb, c0, ncols) in enumerate(pl.chunks2):
                if ncols > 0:
                    gstg = gp.tile([P, 1, ncols], bf16, tag="g")
                    nc.gpsimd.dma_gather(
                        out_ap=gstg[:],
                        in_ap=gsrc,
                        idxs_ap=idx2[:, c0 // 16:(c0 + ncols) // 16],
                        num_idxs=ncols, num_idxs_reg=ncols, elem_size=H,
                        transpose=True, single_packet=False)
                for l in range(l0, l0 + nb):
                    k = int(K2[l])
                    s0 = int(base2[l]) - c0
                    lsl = slice(l * P, (l + 1) * P)
                    mm = ps2.tile([P, P], f32, tag="mm2")
                    for j in range(k):
                        nc.tensor.matmul(
                            out=mm[:], lhsT=W2s[:],
                            rhs=gstg[:, 0, s0 + j * P:s0 + (j + 1) * P],
                            start=(j == 0), stop=False)
                    # appended self-loop term: + W2^T @ u'_local for this block
                    nc.tensor.matmul(
                        out=mm[:], lhsT=W2s[:], rhs=upB[:, lsl],
                        start=(k == 0), stop=True)
                    nc.vector.tensor_tensor(out=u_T[:, lsl], in0=mm[:],
                                            in1=dinv_bc[:, lsl], op=Alu.mult)
                    sqc = tp_.tile([P, P], f32, tag="sqb")
                    nc.scalar.activation(out=sqc[:], in_=u_T[:, lsl],
                                         func=Act.Square, bias=0.0, scale=1.0)
                    nc.vector.tensor_reduce(
                        out=sparts2[:, 2 * l:2 * l + 1], in_=u_T[:, lsl],
                        axis=mybir.AxisListType.X, op=Alu.add)
                    nc.vector.tensor_reduce(
                        out=sparts2[:, 2 * l + 1:2 * l + 2], in_=sqc[:],
                        axis=mybir.AxisListType.X, op=Alu.add)
            ps2.release()
            a2, B2 = finish_stats(sparts2, 1)

            # FC: out = Wfc^T @ relu(a2*u + B2) + bfc. 4-stage pipeline
            # (ACT relu -> PE matmul -> DVE +bias -> DMA out); deep bufs so
            # the 13 chunks stream instead of round-tripping.
            psf = tc.alloc_tile_pool(name="psf", bufs=4, space="PSUM")
            fcp = tc.alloc_tile_pool(name="fcp", bufs=4)
            nch = (SHARD + 511) // 512
            for i in range(nch):
                c0, c1 = i * 512, min((i + 1) * 512, SHARD)
                h2c = fcp.tile([P, c1 - c0], bf16, tag="h2c")
                nc.scalar.activation(out=h2c[:], in_=u_T[:, c0:c1],
                                     func=Act.Relu, bias=B2[:], scale=a2[:])
                fm = psf.tile([O, c1 - c0], f32, tag="fc")
                nc.tensor.matmul(out=fm[:], lhsT=Wfcs[:], rhs=h2c[:],
                                 start=True, stop=True)
                ostg = fcp.tile([O, c1 - c0], f32, tag="ostg")
                nc.vector.tensor_scalar_add(out=ostg[:], in0=fm[:],
                                            scalar1=bfcs[:])
                nc.sync.dma_start(out=out_ext[:, c0:c1], in_=ostg[:])
            fcp.release()
            psf.release()

    nc.finalize()
    return nc


# ----------------------------------------------------------------------------
# Numpy emulation (validates plan + device algorithm quickly)
# ----------------------------------------------------------------------------

def emulate(pl, x, W1, g1, beta1, W2, g2, beta2, Wfc, bfc):
    bf = lambda v: v.astype(ml_dtypes.bfloat16).astype(np.float32)
    maps = per_core_inputs(pl, x)
    W1b, W2f, Wfb = bf(W1), bf(W2), bf(Wfc)
    dinv_pad = np.zeros(NPAD, np.float32)
    mrows = pl.node_of_agrow >= 0
    dinv_pad[mrows] = pl.dinv[pl.node_of_agrow[mrows]]

    # L1 per core
    uT_all = []
    stats1 = np.zeros((P, 2))
    for c in range(NC):
        xe = maps[c]["xeT"].astype(np.float32)   # [128, COLS1]
        agg = np.zeros((P, SHARD), np.float32)
        for g in range(pl.G1):
            w = int(pl.W1g[g])
            kg = int(pl.K1g[g])
            seg = xe[:, int(pl.base1g[g]):int(pl.base1g[g + 1])].reshape(P, kg, w)
            agg[:, g * 4 * P:g * 4 * P + w] = seg.sum(1)
        u = (W1b.T @ agg) * dinv_pad[c * SHARD:(c + 1) * SHARD][None, :]
        uT_all.append(u)
        stats1[:, 0] += u.sum(1)
        stats1[:, 1] += (u * u).sum(1)
    m1 = stats1[:, 0] / N
    v1 = stats1[:, 1] / N - m1 * m1
    a1 = g1 / np.sqrt(v1 + BN_EPS)
    B1 = beta1 - m1 * a1
    gout = np.zeros((NPAD, H), np.float32)
    up_all = []
    for c in range(NC):
        h = np.maximum(a1[:, None] * uT_all[c] + B1[:, None], 0.0)
        up = h * dinv_pad[c * SHARD:(c + 1) * SHARD][None, :]
        gout[c * SHARD:(c + 1) * SHARD] = bf(up.T)
        up_all.append(bf(up))                           # device upB is bf16

    # L2 per core (single window, idx = row - OFF); self-loop term from upB
    u2_all = []
    stats2 = np.zeros((P, 2))
    for c in range(NC):
        idx = pl.idx2[c]
        flat = idx[:16].T.reshape(-1).astype(np.int64) + OFF
        g = gout[flat]                                  # [COLS2, H]
        agg2 = np.zeros((P, SHARD), np.float32)
        for l in range(BPC):
            k = int(pl.K2[l])
            if k:
                seg = g[int(pl.base2[l]):int(pl.base2[l + 1])].reshape(k, P, H)
                agg2[:, l * P:(l + 1) * P] = seg.sum(0).T
        agg2 += up_all[c]
        u2 = (W2f.T @ agg2) * dinv_pad[c * SHARD:(c + 1) * SHARD][None, :]
        u2_all.append(u2)
        stats2[:, 0] += u2.sum(1)
        stats2[:, 1] += (u2 * u2).sum(1)
    m2 = stats2[:, 0] / N
    v2 = stats2[:, 1] / N - m2 * m2
    a2 = g2 / np.sqrt(v2 + BN_EPS)
    B2 = beta2 - m2 * a2
    outpad = np.zeros((NPAD, O), np.float32)
    for c in range(NC):
        h2 = bf(np.maximum(a2[:, None] * u2_all[c] + B2[:, None], 0.0))
        outpad[c * SHARD:(c + 1) * SHARD] = (Wfb.T @ h2 + bfc[:, None]).T
    return outpad[pl.agrow_of_node]


# ----------------------------------------------------------------------------
# Entry point
# ----------------------------------------------------------------------------

_TRACE = [False]
_DEBUG = [False]


def kernel(x, edge_index, W1, b1, g1, beta1, W2, b2, g2, beta2, Wfc, bfc):
    from concourse.bass_utils import run_bass_kernel_spmd

    x = np.asarray(x, np.float32)
    edge_index = np.asarray(edge_index)
    pl = build_plan(edge_index)
    nc = build_device(pl, debug=_DEBUG[0])

    maps = per_core_inputs(pl, x)
    bfc_ = lambda v: np.asarray(v, np.float32).astype(ml_dtypes.bfloat16)
    consts = {
        "W1": bfc_(W1), "W2": bfc_(W2),
        "Wfc": bfc_(Wfc),
        "g1c": np.asarray(g1, np.float32).reshape(H, 1),
        "be1c": np.asarray(beta1, np.float32).reshape(H, 1),
        "g2c": np.asarray(g2, np.float32).reshape(H, 1),
        "be2c": np.asarray(beta2, np.float32).reshape(H, 1),
        "bfcc": np.asarray(bfc, np.float32).reshape(O, 1),
    }
    for mp in maps:
        mp.update(consts)

    res = run_bass_kernel_spmd(
        nc, maps, core_ids=list(range(NC)), trace=_TRACE[0])

    outpad = np.zeros((NPAD, O), np.float32)
    for c in range(NC):
        outpad[c * SHARD:(c + 1) * SHARD] = res.results[c]["out"].T
    out = outpad[pl.agrow_of_node]
    kernel.last_results = res
    return out.astype(np.float32)


# revision 61
# speedup vs baseline: 1.0812x; 1.0812x over previous
"""GCN (2x GCNConv + BN + ReLU + FC) on 8 Trainium2 NeuronCores — v3.

Feature-major dataflow, one bf16 AllGather, single-window L2 gather.

Math: per GCNConv, z[d] = dinv_d * sum_{s->d} dinv_s * h[s] (+bias absorbed
by BN). Structure:
  - L1: columns (edge slots) are host-relayouted from x (edge-ordered,
    dinv_src-scaled, bf16) and streamed; segment-sum fused into the W1
    matmul via PSUM accumulation over slabs.
  - L2: gather bf16 h1' columns out of the AllGathered gout with ONE
    dma_gather window: int16 indices encode (src_row - 17408), exploiting
    sign-extended index address math to cover all 50176 rows.
  - dests sorted by degree within each core so per-128-block max-degree
    padding is minimal; K uniform per block -> one contiguous DVE reduce
    [128, 128, K] per block.
  - BN affine per-partition on ScalarE; BN stats via 1KB AllReduce.

Host does layout/normalization-constant work only (edge ordering, dinv,
tables); all FP reductions/matmuls/BN on device.
"""

import numpy as np
import ml_dtypes

N, E, D, H, O = 50000, 600000, 128, 128, 64
BN_EPS = 1e-5
NC = 8
P = 128
BPC = 49
SHARD = BPC * P          # 6272
NPAD = NC * SHARD        # 50176
OFF = NPAD - 32768       # 17408: idx int16 = row - OFF in [-17408, 32767]
C_COLS = 4096            # target gather cols per dma_gather call


def _wrap_idx(flat):
    flat = np.asarray(flat, np.int16)
    assert flat.size % 16 == 0
    w16 = flat.reshape(-1, 16).T.copy()
    return np.tile(w16, (8, 1))          # [128, n/16]


class Plan:
    pass


def build_plan(edge_index: np.ndarray) -> Plan:
    pl = Plan()
    row = edge_index[0].astype(np.int64)
    col = edge_index[1].astype(np.int64)
    loops = np.arange(N, dtype=np.int64)
    row = np.concatenate([row, loops])
    col = np.concatenate([col, loops])

    deg = np.bincount(col, minlength=N)
    dinv = (1.0 / np.sqrt(deg)).astype(np.float64)
    pl.dinv = dinv.astype(np.float32)

    order = np.argsort(-deg, kind="stable")
    # node -> core (snake deal by degree for edge balance)
    core_of = np.zeros(N, np.int64)
    snake = np.tile(np.r_[np.arange(NC), np.arange(NC)[::-1]],
                    (N + 2 * NC - 1) // (2 * NC))[:N]
    core_of[order] = snake

    # per-core dest ordering: plain degree desc -> tight per-block max-K
    node_of_agrow = np.full(NPAD, -1, np.int64)
    for c in range(NC):
        nodes = np.flatnonzero(core_of == c)
        srt = nodes[np.argsort(-deg[nodes], kind="stable")]
        node_of_agrow[c * SHARD:c * SHARD + len(srt)] = srt
    agrow_of_node = np.full(N, -1, np.int64)
    m = node_of_agrow >= 0
    agrow_of_node[node_of_agrow[m]] = np.flatnonzero(m)
    assert (agrow_of_node >= 0).all()

    # dummy (zero) row with agrow >= OFF for padding slots
    dummies = np.flatnonzero(~m)
    dums_hi = dummies[dummies >= OFF]
    assert len(dums_hi) > 0
    dummy_agrow = int(dums_hi[0])

    # block schedules (uniform across cores):
    #   K1[l] = max WITH-loop degree (L1 xeT slab schedule)
    #   K2[l] = max NON-loop degree (L2 gather schedule) — the appended
    #   self-loop contribution is each core's own u' tile, already resident
    #   feature-major in SBUF; gathering it would waste ~6272 Q7 descriptors
    #   per core. It is added as a second matmul in block_post instead.
    ndeg = np.bincount(edge_index[1].astype(np.int64), minlength=N)
    nodes_blk = node_of_agrow.reshape(NC, BPC, P)
    K1 = np.zeros(BPC, np.int64)
    K2 = np.zeros(BPC, np.int64)
    for c in range(NC):
        for l in range(BPC):
            blk = nodes_blk[c, l]
            real = blk[blk >= 0]
            if len(real):
                K1[l] = max(K1[l], deg[real].max())
                K2[l] = max(K2[l], ndeg[real].max())
    base2 = np.concatenate([[0], np.cumsum(K2 * P)])
    pl.K2, pl.base2 = K2, base2
    pl.COLS2 = int(base2[-1])

    # per-block last-lane guard: the LAST column of each block must encode a
    # non-negative idx (>= OFF source) or the ucode drops it as trailing pad.
    # A lane-127 node with deg < K2[l] ends in a pad (dummy_agrow >= OFF). If
    # every node in a block has deg == K2[l], move a node with a >= OFF
    # source (or pad) to lane 127 and put that source last.
    # We handle this after edge tables are built (cheap per-block fixup).

    # L1 groups of 4 blocks sharing one 512-wide PSUM accumulation chain
    G1 = (BPC + 3) // 4
    K1g = np.array([int(K1[4 * g:4 * g + 4].max()) for g in range(G1)])
    W1g = np.array([min(4, BPC - 4 * g) * P for g in range(G1)])
    base1g = np.concatenate([[0], np.cumsum(K1g * W1g)])
    pl.G1, pl.K1g, pl.W1g, pl.base1g = G1, K1g, W1g, base1g
    pl.COLS1 = int(base1g[-1])

    # per-agrow column bases
    lidx = np.arange(NPAD) % SHARD // P              # block l of each agrow
    pidx = np.arange(NPAD) % P
    gidx = lidx // 4
    bidx = lidx % 4
    cb1 = base1g[gidx] + bidx * P + pidx             # L1 col base per agrow
    w1 = W1g[gidx]                                   # L1 col stride per agrow
    cb2 = base2[lidx] + pidx * K2[lidx]              # L2 col base per agrow

    # per-edge (L1, WITH self-loops): dest agrow, source node, rank
    e_dst = agrow_of_node[col]
    o2 = np.argsort(e_dst, kind="stable")
    d_s = e_dst[o2]
    src_node = row[o2]
    newd = np.r_[True, d_s[1:] != d_s[:-1]]
    gstart = np.flatnonzero(newd)
    gid = np.cumsum(newd) - 1
    k_all = np.arange(len(d_s)) - gstart[gid]        # rank within dest
    core_d = d_s // SHARD

    # per-edge (L2, WITHOUT the appended self-loops)
    rowE = edge_index[0].astype(np.int64)
    colE = edge_index[1].astype(np.int64)
    eE_dst = agrow_of_node[colE]
    oE = np.argsort(eE_dst, kind="stable")
    dE = eE_dst[oE]
    srcagE = agrow_of_node[rowE[oE]]
    newdE = np.r_[True, dE[1:] != dE[:-1]]
    gstartE = np.flatnonzero(newdE)
    gidE = np.cumsum(newdE) - 1
    kE = np.arange(len(dE)) - gstartE[gidE]
    coreE = dE // SHARD

    pl.xe_src = []
    pl.idx2 = []
    for c in range(NC):
        mC = core_d == c
        dC, kC = d_s[mC], k_all[mC]
        xs = np.full(pl.COLS1, -1, np.int64)
        xs[cb1[dC] + kC * w1[dC]] = src_node[mC]
        mE = coreE == c
        ix = np.full(pl.COLS2, dummy_agrow - OFF, np.int64)
        ix[cb2[dE[mE]] + kE[mE]] = srcagE[mE] - OFF
        # last-column guard per block
        for l in range(BPC):
            k = int(K2[l])
            if k == 0:
                continue
            lastcol = int(base2[l]) + 127 * k + (k - 1)
            if ix[lastcol] >= 0:
                continue
            # try to swap within lane 127's slot range
            lo = int(base2[l]) + 127 * k
            seg = ix[lo:lo + k]
            pos = np.flatnonzero(seg >= 0)
            if len(pos):
                j = int(pos[-1])
                seg[k - 1], seg[j] = seg[j], seg[k - 1]
                ix[lo:lo + k] = seg
                # xe table is unaffected (L1 slot order independent)
                continue
            raise AssertionError(
                f"core {c} block {l}: no non-negative idx for last column")
        assert ix[int(base2[BPC]) - 1] >= 0 if False else True
        pl.xe_src.append(xs)
        pl.idx2.append(_wrap_idx(ix))

    # gather call chunking: (start_block, n_blocks, col0, ncols)
    out = []
    l = 0
    while l < BPC:
        l0, c0 = l, int(base2[l])
        l += 1                              # always take one block
        while l < BPC and int(base2[l + 1]) - c0 <= C_COLS:
            l += 1
        out.append((l0, l - l0, c0, int(base2[l]) - c0))
    pl.chunks2 = out
    pl.MAXCH = max(nc_ for (_, _, _, nc_) in out)

    pl.node_of_agrow = node_of_agrow
    pl.agrow_of_node = agrow_of_node
    return pl


def per_core_inputs(pl: Plan, x: np.ndarray) -> list[dict]:
    xs_pre = (x.astype(np.float64) * pl.dinv.astype(np.float64)[:, None])
    xs_pre = xs_pre.astype(np.float32)
    maps = []
    for c in range(NC):
        src = pl.xe_src[c]
        xe = np.zeros((pl.COLS1, D), np.float32)
        mm = src >= 0
        xe[mm] = xs_pre[src[mm]]
        dv = np.zeros(SHARD, np.float32)
        nodes = pl.node_of_agrow[c * SHARD:(c + 1) * SHARD]
        mn = nodes >= 0
        dv[mn] = pl.dinv[nodes[mn]]
        maps.append({
            "xeT": np.ascontiguousarray(xe.T).astype(ml_dtypes.bfloat16),
            "idx2": pl.idx2[c],
            "dinvb": dv.reshape(1, SHARD),
        })
    return maps


# ----------------------------------------------------------------------------
# Device program
# ----------------------------------------------------------------------------

def build_device(pl: Plan, debug=False):
    import concourse.bacc as bacc
    import concourse.mybir as mybir
    import concourse.tile as tile
    from concourse.masks import make_identity

    f32 = mybir.dt.float32
    bf16 = mybir.dt.bfloat16
    i16 = mybir.dt.int16
    Alu = mybir.AluOpType
    Act = mybir.ActivationFunctionType

    K2, base2 = pl.K2, pl.base2
    G1, K1g, W1g, base1g = pl.G1, pl.K1g, pl.W1g, pl.base1g
    COLS1, COLS2 = pl.COLS1, pl.COLS2

    nc = bacc.Bacc()

    xeT_in = nc.declare_dram_parameter("xeT", [P, COLS1], bf16, isOutput=False)
    idx2_in = nc.declare_dram_parameter("idx2", [P, COLS2 // 16], i16, isOutput=False)
    dinvb_in = nc.declare_dram_parameter("dinvb", [1, SHARD], f32, isOutput=False)
    W1_in = nc.declare_dram_parameter("W1", [D, H], bf16, isOutput=False)
    W2_in = nc.declare_dram_parameter("W2", [H, H], bf16, isOutput=False)
    Wfc_in = nc.declare_dram_parameter("Wfc", [H, O], bf16, isOutput=False)
    g1_in = nc.declare_dram_parameter("g1c", [H, 1], f32, isOutput=False)
    be1_in = nc.declare_dram_parameter("be1c", [H, 1], f32, isOutput=False)
    g2_in = nc.declare_dram_parameter("g2c", [H, 1], f32, isOutput=False)
    be2_in = nc.declare_dram_parameter("be2c", [H, 1], f32, isOutput=False)
    bfc_in = nc.declare_dram_parameter("bfcc", [O, 1], f32, isOutput=False)
    out_ext = nc.declare_dram_parameter("out", [O, SHARD], f32, isOutput=True)

    ag_in = nc.dram_tensor("ag_in", [SHARD, H], bf16)
    gout = nc.dram_tensor("gout", [NPAD, H], bf16, addr_space="Shared")
    ar_in = [nc.dram_tensor(f"ar_in{i}", [P, 2], f32) for i in range(2)]
    ar_out = [nc.dram_tensor(f"ar_out{i}", [P * NC, 2], f32, addr_space="Shared")
              for i in range(2)]
    rg = [list(range(NC))]

    with tile.TileContext(nc) as tc:
        with (
            tc.tile_pool(name="const", bufs=1) as cp,
            tc.tile_pool(name="xstg", bufs=2) as xp,
            tc.tile_pool(name="gstg", bufs=4) as gp,
            tc.tile_pool(name="big", bufs=1) as bigp,
            tc.tile_pool(name="tmp", bufs=2) as tp_,
            tc.tile_pool(name="scal", bufs=1) as sp,
        ):
            # ------- constants needed for L1 start (W1, idx2, dinv) -------
            W1s = cp.tile([D, H], bf16, tag="W1")
            nc.sync.dma_start(out=W1s[:], in_=W1_in[:])
            idx2 = cp.tile([P, COLS2 // 16], i16, tag="idx2")
            nc.sync.dma_start(out=idx2[:], in_=idx2_in[:])
            dinv_bc = cp.tile([P, SHARD], f32, tag="dinvbc")
            nc.sync.dma_start(out=dinv_bc[:], in_=dinvb_in[:].to_broadcast((P, SHARD)))

            u_T = bigp.tile([P, SHARD], f32, tag="u_T", name="u_T")
            gsrc = gout[OFF:OFF + 32768, :]

            # ------- gather-ucode warmup -------
            # The first dma_gather pays a ~10us ext-isa IRAM library load on
            # GpSimd. Issue a tiny throwaway gather NOW (gout content is
            # garbage pre-AllGather; addresses are in-bounds, output unused)
            # so the reload happens off the critical path.
            warm = sp.tile([P, 1, P], bf16, tag="warm")
            nc.gpsimd.dma_gather(
                out_ap=warm[:], in_ap=gsrc,
                idxs_ap=idx2[:, 0:8],
                num_idxs=P, num_idxs_reg=P, elem_size=H,
                transpose=True, single_packet=False)

            # ------- L1: fused aggregation+matmul via PSUM accumulation ----
            ps1 = tc.alloc_tile_pool(name="ps1", bufs=3, space="PSUM")
            SLABS_PER_STG = 16
            for g in range(G1):
                w = int(W1g[g])
                kg = int(K1g[g])
                c0 = int(base1g[g])
                mm = ps1.tile([P, w], f32, tag="mm1")
                for ks in range(0, kg, SLABS_PER_STG):
                    kn = min(SLABS_PER_STG, kg - ks)
                    xstg = xp.tile([P, SLABS_PER_STG * w], bf16, tag="xe")
                    nc.sync.dma_start(
                        out=xstg[:, 0:kn * w],
                        in_=xeT_in[:, c0 + ks * w:c0 + (ks + kn) * w])
                    for k in range(kn):
                        nc.tensor.matmul(
                            out=mm[:], lhsT=W1s[:],
                            rhs=xstg[:, k * w:(k + 1) * w],
                            start=(ks + k == 0), stop=(ks + k == kg - 1))
                o0 = g * 4 * P
                nc.vector.tensor_tensor(
                    out=u_T[:, o0:o0 + w], in0=mm[:], in1=dinv_bc[:, o0:o0 + w],
                    op=Alu.mult)
            ps1.release()

            # ------- remaining constants (first used after L1) -------
            W2s = cp.tile([H, H], bf16, tag="W2")
            Wfcs = cp.tile([H, O], bf16, tag="Wfc")
            nc.sync.dma_start(out=W2s[:], in_=W2_in[:])
            nc.sync.dma_start(out=Wfcs[:], in_=Wfc_in[:])
            gb = {}
            for nm, t in [("g1", g1_in), ("be1", be1_in),
                          ("g2", g2_in), ("be2", be2_in)]:
                gb[nm] = cp.tile([H, 1], f32, tag=nm, name=nm + "_sb")
                nc.sync.dma_start(out=gb[nm][:], in_=t[:])
            bfcs = cp.tile([O, 1], f32, tag="bfc")
            nc.sync.dma_start(out=bfcs[:], in_=bfc_in[:])
            ident = cp.tile([P, P], f32, tag="ident")
            make_identity(nc, ident[:])

            # BN stats helpers: per-chunk partials -> AllReduce -> affine
            def stats_chunk(sparts, i, c0, c1):
                sqc = tp_.tile([P, c1 - c0], f32, tag="sqc")
                nc.scalar.activation(out=sqc[:], in_=u_T[:, c0:c1],
                                     func=Act.Square, bias=0.0, scale=1.0)
                nc.vector.tensor_reduce(
                    out=sparts[:, 2 * i:2 * i + 1], in_=u_T[:, c0:c1],
                    axis=mybir.AxisListType.X, op=Alu.add)
                nc.vector.tensor_reduce(
                    out=sparts[:, 2 * i + 1:2 * i + 2], in_=sqc[:],
                    axis=mybir.AxisListType.X, op=Alu.add)

            def finish_stats(sparts, li):
                ssb = sp.tile([P, 2], f32, tag=f"ssb{li}")
                nc.vector.tensor_reduce(
                    out=ssb[:], in_=sparts[:].rearrange("p (c s) -> p s c", s=2),
                    axis=mybir.AxisListType.X, op=Alu.add)
                nc.sync.dma_start(out=ar_in[li][:], in_=ssb[:])
                # stats exchange as a tiny AllGather (lower floor + less
                # variance than AllReduce's reduce+broadcast phases); the
                # 8-shard sum is done locally on DVE.
                nc.gpsimd.collective_compute(
                    "AllGather", Alu.bypass, replica_groups=rg,
                    ins=[ar_in[li][:]], outs=[ar_out[li][:]])
                sall = sp.tile([P, 2 * NC], f32, tag=f"sall{li}")
                nc.sync.dma_start(
                    out=sall[:].rearrange("p (r s) -> p r s", s=2),
                    in_=ar_out[li][:].rearrange("(r p) s -> p r s", p=P))
                sums = sp.tile([P, 2], f32, tag=f"sums{li}")
                nc.vector.tensor_reduce(
                    out=sums[:],
                    in_=sall[:].rearrange("p (r s) -> p s r", s=2),
                    axis=mybir.AxisListType.X, op=Alu.add)
                mom = sp.tile([P, 2], f32, tag=f"mom{li}")
                nc.vector.tensor_scalar_mul(out=mom[:], in0=sums[:],
                                            scalar1=1.0 / N)
                msq = sp.tile([P, 1], f32, tag=f"msq{li}")
                nc.vector.tensor_tensor(out=msq[:], in0=mom[:, 0:1],
                                        in1=mom[:, 0:1], op=Alu.mult)
                v = sp.tile([P, 1], f32, tag=f"v{li}")
                nc.vector.tensor_tensor(out=v[:], in0=mom[:, 1:2], in1=msq[:],
                                        op=Alu.subtract)
                nc.vector.tensor_scalar_add(out=v[:], in0=v[:], scalar1=BN_EPS)
                sqv = sp.tile([P, 1], f32, tag=f"sqv{li}")
                nc.scalar.activation(out=sqv[:], in_=v[:], func=Act.Sqrt,
                                     bias=0.0, scale=1.0)
                rsq = sp.tile([P, 1], f32, tag=f"rsq{li}")
                nc.vector.reciprocal(out=rsq[:], in_=sqv[:])
                g, be = (gb["g1"], gb["be1"]) if li == 0 else (gb["g2"], gb["be2"])
                a = sp.tile([P, 1], f32, tag=f"a{li}")
                nc.vector.tensor_tensor(out=a[:], in0=rsq[:], in1=g[:], op=Alu.mult)
                ma = sp.tile([P, 1], f32, tag=f"ma{li}")
                nc.vector.tensor_tensor(out=ma[:], in0=mom[:, 0:1], in1=a[:],
                                        op=Alu.mult)
                B = sp.tile([P, 1], f32, tag=f"B{li}")
                nc.vector.tensor_tensor(out=B[:], in0=be[:], in1=ma[:],
                                        op=Alu.subtract)
                return a, B

            # L1 stats (u_T already computed by the fused PSUM path)
            nchs = (SHARD + 2047) // 2048
            sparts1 = sp.tile([P, 2 * nchs], f32, tag="sp0")
            for i in range(nchs):
                stats_chunk(sparts1, i, i * 2048, min((i + 1) * 2048, SHARD))
            a1, B1 = finish_stats(sparts1, 0)

            # h1 = relu(a*u + B); u' = h1 * dinv; transpose; ship — all
            # pipelined per 512-column chunk. upT aliases agg2's slot.
            upT = bigp.tile([P, SHARD], f32, tag="agg2", name="upT")
            upB = bigp.tile([P, SHARD], bf16, tag="upB", name="upB")
            pst = tc.alloc_tile_pool(name="pst", bufs=6, space="PSUM")
            for l0 in range(0, BPC, 4):
                nb = min(4, BPC - l0)
                csl = slice(l0 * P, (l0 + nb) * P)
                nc.scalar.activation(out=upT[:, csl], in_=u_T[:, csl],
                                     func=Act.Relu, bias=B1[:], scale=a1[:])
                nc.vector.tensor_tensor(out=upT[:, csl], in0=upT[:, csl],
                                        in1=dinv_bc[:, csl], op=Alu.mult)
                nc.vector.tensor_copy(out=upB[:, csl], in_=upT[:, csl])
                stg = tp_.tile([P, nb * P], bf16, tag="upstg")
                for j in range(nb):
                    l = l0 + j
                    tpm = pst.tile([P, P], f32, tag="tp")
                    nc.tensor.transpose(
                        out=tpm[:], in_=upT[:, l * P:(l + 1) * P],
                        identity=ident[:])
                    nc.scalar.activation(out=stg[:, j * P:(j + 1) * P],
                                         in_=tpm[:], func=Act.Copy,
                                         bias=0.0, scale=1.0)
                nc.sync.dma_start(
                    out=ag_in[l0 * P:(l0 + nb) * P, :].rearrange(
                        "(b p) q -> p b q", b=nb),
                    in_=stg[:].rearrange("p (b q) -> p b q", b=nb))
            pst.release()
            nc.gpsimd.collective_compute(
                "AllGather", Alu.bypass, replica_groups=rg,
                ins=[ag_in[:]], outs=[gout[:]])
            probe = sp.tile([1, H], bf16, tag="probe")
            nc.sync.dma_start(out=probe[:], in_=gout[0:1, :])

            # ---------------- L2 gather + aggregation ----------------------
            agg2 = bigp.tile([P, SHARD], f32, tag="agg2", name="agg2")
            sparts2 = sp.tile([P, 2 * BPC], f32, tag="sp1")
            ps2 = tc.alloc_tile_pool(name="ps2", bufs=2, space="PSUM")

            def block_post(l, k):
                # W2 matmul on gathered aggregate + local self-loop term
                # (u'_d for each dest d of the block, from upB), dinv, stats
                lsl = slice(l * P, (l + 1) * P)
                mm = ps2.tile([P, P], f32, tag="mm2")
                if k > 0:
                    ab = tp_.tile([P, P], bf16, tag="aggb")
                    nc.vector.tensor_copy(out=ab[:], in_=agg2[:, lsl])
                    nc.tensor.matmul(out=mm[:], lhsT=W2s[:], rhs=ab[:],
                                     start=True, stop=False)
                nc.tensor.matmul(out=mm[:], lhsT=W2s[:], rhs=upB[:, lsl],
                                 start=(k == 0), stop=True)
                nc.vector.tensor_tensor(out=u_T[:, lsl], in0=mm[:],
                                        in1=dinv_bc[:, lsl], op=Alu.mult)
                sqc = tp_.tile([P, P], f32, tag="sqb")
                nc.scalar.activation(out=sqc[:], in_=u_T[:, lsl],
                                     func=Act.Square, bias=0.0, scale=1.0)
                nc.vector.tensor_reduce(
                    out=sparts2[:, 2 * l:2 * l + 1], in_=u_T[:, lsl],
                    axis=mybir.AxisListType.X, op=Alu.add)
                nc.vector.tensor_reduce(
                    out=sparts2[:, 2 * l + 1:2 * l + 2], in_=sqc[:],
                    axis=mybir.AxisListType.X, op=Alu.add)

            for ci, (l0, nb, c0, ncols) in enumerate(pl.chunks2):
                if ncols > 0:
                    gstg = gp.tile([P, 1, ncols], bf16, tag="g")
                    nc.gpsimd.dma_gather(
                        out_ap=gstg[:],
                        in_ap=gsrc,
                        idxs_ap=idx2[:, c0 // 16:(c0 + ncols) // 16],
                        num_idxs=ncols, num_idxs_reg=ncols, elem_size=H,
                        transpose=True, single_packet=False)
                for l in range(l0, l0 + nb):
                    k = int(K2[l])
                    if k > 0:
                        s0 = int(base2[l]) - c0
                        seg = gstg[:, 0, s0:s0 + P * k].rearrange(
                            "p (d k) -> p d k", k=k)
                        nc.vector.tensor_reduce(
                            out=agg2[:, l * P:(l + 1) * P], in_=seg,
                            axis=mybir.AxisListType.X, op=Alu.add)
                    block_post(l, k)
            ps2.release()
            a2, B2 = finish_stats(sparts2, 1)

            # FC: out = Wfc^T @ relu(a2*u + B2) + bfc. 4-stage pipeline
            # (ACT relu -> PE matmul -> DVE +bias -> DMA out); deep bufs so
            # the 13 chunks stream instead of round-tripping.
            psf = tc.alloc_tile_pool(name="psf", bufs=4, space="PSUM")
            fcp = tc.alloc_tile_pool(name="fcp", bufs=4)
            nch = (SHARD + 511) // 512
            for i in range(nch):
                c0, c1 = i * 512, min((i + 1) * 512, SHARD)
                h2c = fcp.tile([P, c1 - c0], bf16, tag="h2c")
                nc.scalar.activation(out=h2c[:], in_=u_T[:, c0:c1],
                                     func=Act.Relu, bias=B2[:], scale=a2[:])
                fm = psf.tile([O, c1 - c0], f32, tag="fc")
                nc.tensor.matmul(out=fm[:], lhsT=Wfcs[:], rhs=h2c[:],
                                 start=True, stop=True)
                ostg = fcp.tile([O, c1 - c0], f32, tag="ostg")
                nc.vector.tensor_scalar_add(out=ostg[:], in0=fm[:],
                                            scalar1=bfcs[:])
                nc.sync.dma_start(out=out_ext[:, c0:c1], in_=ostg[:])
            fcp.release()
            psf.release()

    nc.finalize()
    return nc


# ----------------------------------------------------------------------------
# Numpy emulation (validates plan + device algorithm quickly)
# ----------------------------------------------------------------------------

def emulate(pl, x, W1, g1, beta1, W2, g2, beta2, Wfc, bfc):
    bf = lambda v: v.astype(ml_dtypes.bfloat16).astype(np.float32)
    maps = per_core_inputs(pl, x)
    W1b, W2f, Wfb = bf(W1), bf(W2), bf(Wfc)
    dinv_pad = np.zeros(NPAD, np.float32)
    mrows = pl.node_of_agrow >= 0
    dinv_pad[mrows] = pl.dinv[pl.node_of_agrow[mrows]]

    # L1 per core
    uT_all = []
    stats1 = np.zeros((P, 2))
    for c in range(NC):
        xe = maps[c]["xeT"].astype(np.float32)   # [128, COLS1]
        agg = np.zeros((P, SHARD), np.float32)
        for g in range(pl.G1):
            w = int(pl.W1g[g])
            kg = int(pl.K1g[g])
            seg = xe[:, int(pl.base1g[g]):int(pl.base1g[g + 1])].reshape(P, kg, w)
            agg[:, g * 4 * P:g * 4 * P + w] = seg.sum(1)
        u = (W1b.T @ agg) * dinv_pad[c * SHARD:(c + 1) * SHARD][None, :]
        uT_all.append(u)
        stats1[:, 0] += u.sum(1)
        stats1[:, 1] += (u * u).sum(1)
    m1 = stats1[:, 0] / N
    v1 = stats1[:, 1] / N - m1 * m1
    a1 = g1 / np.sqrt(v1 + BN_EPS)
    B1 = beta1 - m1 * a1
    gout = np.zeros((NPAD, H), np.float32)
    up_all = []
    for c in range(NC):
        h = np.maximum(a1[:, None] * uT_all[c] + B1[:, None], 0.0)
        up = h * dinv_pad[c * SHARD:(c + 1) * SHARD][None, :]
        gout[c * SHARD:(c + 1) * SHARD] = bf(up.T)
        up_all.append(bf(up))                           # device upB is bf16

    # L2 per core (single window, idx = row - OFF); self-loop term from upB
    u2_all = []
    stats2 = np.zeros((P, 2))
    for c in range(NC):
        idx = pl.idx2[c]
        flat = idx[:16].T.reshape(-1).astype(np.int64) + OFF
        g = gout[flat]                                  # [COLS2, H]
        agg2 = np.zeros((P, SHARD), np.float32)
        for l in range(BPC):
            k = int(pl.K2[l])
            if k:
                seg = g[int(pl.base2[l]):int(pl.base2[l + 1])].reshape(P, k, H)
                agg2[:, l * P:(l + 1) * P] = seg.sum(1).T
        agg2 = bf(agg2) + up_all[c]                     # device agg2 is bf16
        u2 = (W2f.T @ agg2) * dinv_pad[c * SHARD:(c + 1) * SHARD][None, :]
        u2_all.append(u2)
        stats2[:, 0] += u2.sum(1)
        stats2[:, 1] += (u2 * u2).sum(1)
    m2 = stats2[:, 0] / N
    v2 = stats2[:, 1] / N - m2 * m2
    a2 = g2 / np.sqrt(v2 + BN_EPS)
    B2 = beta2 - m2 * a2
    outpad = np.zeros((NPAD, O), np.float32)
    for c in range(NC):
        h2 = bf(np.maximum(a2[:, None] * u2_all[c] + B2[:, None], 0.0))
        outpad[c * SHARD:(c + 1) * SHARD] = (Wfb.T @ h2 + bfc[:, None]).T
    return outpad[pl.agrow_of_node]


# ----------------------------------------------------------------------------
# Entry point
# ----------------------------------------------------------------------------

_TRACE = [False]
_DEBUG = [False]


def kernel(x, edge_index, W1, b1, g1, beta1, W2, b2, g2, beta2, Wfc, bfc):
    from concourse.bass_utils import run_bass_kernel_spmd

    x = np.asarray(x, np.float32)
    edge_index = np.asarray(edge_index)
    pl = build_plan(edge_index)
    nc = build_device(pl, debug=_DEBUG[0])

    maps = per_core_inputs(pl, x)
    bfc_ = lambda v: np.asarray(v, np.float32).astype(ml_dtypes.bfloat16)
    consts = {
        "W1": bfc_(W1), "W2": bfc_(W2),
        "Wfc": bfc_(Wfc),
        "g1c": np.asarray(g1, np.float32).reshape(H, 1),
        "be1c": np.asarray(beta1, np.float32).reshape(H, 1),
        "g2c": np.asarray(g2, np.float32).reshape(H, 1),
        "be2c": np.asarray(beta2, np.float32).reshape(H, 1),
        "bfcc": np.asarray(bfc, np.float32).reshape(O, 1),
    }
    for mp in maps:
        mp.update(consts)

    res = run_bass_kernel_spmd(
        nc, maps, core_ids=list(range(NC)), trace=_TRACE[0])

    outpad = np.zeros((NPAD, O), np.float32)
    for c in range(NC):
        outpad[c * SHARD:(c + 1) * SHARD] = res.results[c]["out"].T
    out = outpad[pl.agrow_of_node]
    kernel.last_results = res
    return out.astype(np.float32)
